# revision 65
# baseline (speedup 1.0000x reference)
"""Tied-row (MSA) attention on 8 Trainium2 NeuronCores.

Reference computation (B=128, n=512, dim=256, h=8, dh=64,
r=tie_attn_dim=64, b=B//r=2):
    q = x @ Wq ; k,v = split(x @ Wkv)
    dots[b,h,i,j] = sum_{r,d} q[b,r,h,i,d] k[b,r,h,j,d] * scale
    attn = softmax_j(dots)
    out[b,r,h,i,d] = sum_j attn[b,h,i,j] v[b,r,h,j,d]
    y = out @ Wo + bo
Sharding: 8 cores = b(2) x head-pairs(4).  Each core owns one batch
element and 2 of the 8 heads and produces the f16 partial
    y_part = out[:, :, own 2 heads, :] @ Wo[own 128 rows, :]
The host sums the 4 partials per b and adds bo.

Design notes (driven by the TimelineSim cost model):
- A matmul is charged free_size(out) cycles regardless of contraction
  depth K or output-partition count P, so every matmul is arranged as
  K=128 x P=128: total PE time is the MAC floor, 524288 rows ~= 219 us
  (the naive K=64/P=64 layout costs 786432).
- The K=128 contractions need pair-packed layouts [(par,d), ...] where
  par = r&1: built by repacking copies.  GPSIMD cannot touch PSUM, so
  each PSUM result is staged once to f16 SBUF (ACT/DVE), then repacked
  with f16 SBUF->SBUF copies (DVE runs those at 4x; Pool is legal too).
- Per-tile single-writer-engine discipline everywhere: a second writer
  on a different engine serializes behind the first (cross-engine WAW
  in Tile's tracker), which otherwise poisons the WAR chains.

Layouts (pair = r>>1):
  q20/k20[m]  [128=(par,d), 16, 512] f16, m = pair%2 (parity-split so
              the fused dots' tile-granular RAW dep is 2 pairs stale)
  q21/k21[m]  [128=(par,d), 8, 512] f16, m = pair%4 (4-way: the h1 dots
              run 4 pairs behind; their k21 supply chain is DVE->Pool)
  dots[h,it]  [128 i, 512 j] PSUM, accumulated over all 32 pairs
  attnT[h]    [128 j, 4 jc, 512 i] f16 via xbar DMA transposes
  v2h[h]      [128 j, 4 jc, 128=(par,d)] f16 per pair
  o[h]        [128=(par,d), 512 i] PSUM = sum_jc v2h^T @ attnT (K=128)
  yin[par]    [128=(h,d), 512 i] f16 (f16 splits regroup (par,d)->(h,d))
  y_ps        [128 i, 4 it, 256 e] PSUM = yin^T @ wo (K=128)

Phase-A engine map (per r, 1278ns PE budget): ACT: q_st stage + q20
split on even pairs; DVE: k_st stage + q21/k20 splits (+q20 odd pairs);
Pool: k21 split.  Phase-C map (per pair, 3413ns): ACT: vst stages +
ost0 + ysb0; DVE: ost1 + yin splits + ysb1; Pool: v2h splits.

Phases: A) r-loop: q/k proj, stage, split; dots h0 it0/1 fused two
pairs behind and h1 it0/1 four pairs behind (PSUM: psv 2 + d0 2 + d1a 2
+ qk 2).  B1) dots h0 it2/3 + early softmaxes; B2) dots h1 it2/3 + the
rest; phase-C x blocks prefetch during B; psv keeps v_ps banks free of
any softmax dependency.  C) pair-loop SW pipeline a=v-proj (2 ahead),
b=out2+yin, c=y+DMA (last-pair drain split across engines/queues).

Built with bacc.Bacc(): its compile() pass legalizes Tile's sync for
walrus; callers must finalize() before running (see _get_program).
"""

import os
import sys

for _p in ("/opt/trn_rl_repo", "/root/.axon_site/_ro/trn_rl_repo"):
    if os.path.isdir(_p) and _p not in sys.path:
        sys.path.insert(0, _p)

import numpy as np

R = 64          # tie dim (MSA rows per batch element)
NPAIR = 32      # r-pairs
RB = 4          # rows per x DMA block
NPREF = 3       # phase-C x blocks prefetched before phase B
N = 512         # sequence length
C = 256         # model dim
HP = 128        # head-pair width: 2 heads x 64
E = 256         # output dim
NCORES = 8

_CACHE = {}


def build_program(phases=(1, 2, 3)):
    import concourse.bacc as bacc
    from concourse import mybir
    from concourse.tile import TileContext
    from contextlib import ExitStack

    f32 = mybir.dt.float32
    f16 = mybir.dt.float16

    nc = bacc.Bacc()
    xT = nc.declare_dram_parameter("xT", [R, C, N], f16, isOutput=False)
    wq = nc.declare_dram_parameter("wq", [C, HP], f16, isOutput=False)
    wk = nc.declare_dram_parameter("wk", [C, HP], f16, isOutput=False)
    wv = nc.declare_dram_parameter("wv", [C, HP], f16, isOutput=False)
    wo = nc.declare_dram_parameter("wo", [HP, E], f16, isOutput=False)
    y = nc.declare_dram_parameter("y", [R, N, E], f16, isOutput=True)

    # x block rb viewed as [p, r_in_block, c_chunk, n]
    xT_blk = xT.rearrange("(rb r) (cc p) n -> rb p r cc n", r=RB, p=128)
    # y block per pair: [p=i, par, i_tile, e]
    y_blk = y.rearrange("(pair par) (t p) e -> pair p par t e", par=2, p=128)

    with TileContext(nc) as tc, ExitStack() as ctx:
        singles = ctx.enter_context(tc.tile_pool(name="singles", bufs=1))
        sm = ctx.enter_context(tc.tile_pool(name="sm", bufs=4))
        attnp = ctx.enter_context(tc.tile_pool(name="attnp", bufs=4))
        attntp = ctx.enter_context(tc.tile_pool(name="attntp", bufs=2))
        xpool = ctx.enter_context(tc.tile_pool(name="xpool", bufs=NPREF))
        stg = ctx.enter_context(tc.tile_pool(name="stg", bufs=2))
        resid = ctx.enter_context(tc.tile_pool(name="resid", bufs=1))

        # first x rows go out before anything else so phase A starts ASAP;
        # row blocks 1|3|4|4|... so the first q matmul waits on a 1-row DMA
        def x_dma(engine, tile, r0, nr):
            engine.dma_start(
                out=tile,
                in_=xT[r0:r0 + nr].rearrange("r (cc p) n -> p r cc n", p=128))

        xa0 = xpool.tile([128, 1, 2, N], f16, tag="x", name="x1_h0")
        x_dma(nc.sync, xa0, 0, 1)

        # weights: [256, X] -> sbuf [128, 2, X]; one DMA each on the HWDGE
        # queues, emitted after the first x rows so they queue behind them
        # on DMA_ENGINES
        wq_sb = singles.tile([128, 2, HP], f16)
        wk_sb = singles.tile([128, 2, HP], f16)
        wv_sb = singles.tile([128, 2, HP], f16)
        wo_sb = singles.tile([128, E], f16)
        nc.scalar.dma_start(out=wq_sb, in_=wq.rearrange("(cc p) h -> p cc h", p=128))
        xa1 = xpool.tile([128, 3, 2, N], f16, tag="x", name="x1_h1")
        x_dma(nc.scalar, xa1, 1, 3)
        nc.sync.dma_start(out=wk_sb, in_=wk.rearrange("(cc p) h -> p cc h", p=128))
        nc.scalar.dma_start(out=wv_sb, in_=wv.rearrange("(cc p) h -> p cc h", p=128))
        nc.sync.dma_start(out=wo_sb, in_=wo[:, :])

        # pair-packed projections, resident through phases A+B
        q20 = [resid.tile([128, NPAIR // 2, N], f16, name=f"q20_{m}")
               for m in range(2)]
        k20 = [resid.tile([128, NPAIR // 2, N], f16, name=f"k20_{m}")
               for m in range(2)]
        q21 = [resid.tile([128, NPAIR // 4, N], f16, name=f"q21_{m}")
               for m in range(4)]
        k21 = [resid.tile([128, NPAIR // 4, N], f16, name=f"k21_{m}")
               for m in range(4)]
        # attnT survives into phase C
        attnT = [attntp.tile([128, 4, N], f16, tag="attnT", name=f"attnT_{h}")
                 for h in range(2)]

        def softmax(dots_hit, h, it):
            """dots PSUM tile -> normalized f16 attn SBUF tile.

            No max-subtraction: dots = q k^T with the 1/(sqrt(dh) sqrt(r))
            scale folded into Wq, so entries are ~N(0,1) and exp cannot
            overflow fp32/fp16."""
            ssum = sm.tile([128, 1], f32, tag="ssum", bufs=8)
            rinv = sm.tile([128, 1], f32, tag="rinv", bufs=8)
            attn = attnp.tile([128, N], f16, tag="attn", bufs=4,
                              name=f"attn_{h}_{it}")
            nc.scalar.activation(
                out=attn, in_=dots_hit,
                func=mybir.ActivationFunctionType.Exp,
                accum_out=ssum)
            nc.vector.reciprocal(rinv, ssum)
            nc.vector.tensor_scalar_mul(attn, attn, rinv)
            return attn

        def transpose_attn(attn, h, it):
            # one f16 xbar DMA transpose, SBUF -> SBUF: out[j, jc, i] =
            # attn[i, jc*128 + j]
            nc.sync.dma_start_transpose(
                out=attnT[h][:, :, it * 128:(it + 1) * 128], in_=attn)

        dots = {}

        def dots_h0(p, its):
            qq, kk, j = q20[p % 2], k20[p % 2], p // 2
            for it in its:
                nc.tensor.matmul(
                    dots[(0, it)],
                    lhsT=qq[:, j, it * 128:(it + 1) * 128], rhs=kk[:, j, :],
                    start=(p == 0), stop=(p == NPAIR - 1))

        def dots_h1(p, its):
            qq, kk, j = q21[p % 4], k21[p % 4], p // 4
            for it in its:
                nc.tensor.matmul(
                    dots[(1, it)],
                    lhsT=qq[:, j, it * 128:(it + 1) * 128], rhs=kk[:, j, :],
                    start=(p == 0), stop=(p == NPAIR - 1))

        # PSUM plan (8 banks, LIFO): psv(2, v_ps — reserved from t=0 so
        # phase-C v-projs never wait on a softmax) > psd0(2, dots h0 it0/1)
        # > psd1a(2, dots h1 it0/1) > [psqk(2) | psB1(2, h0 it2/3) |
        # psB2(2, h1 it2/3)]; after popping psB2/psd1a/psd0: psc(6).
        psv = ctx.enter_context(tc.tile_pool(name="psv", space="PSUM", bufs=2))
        with tc.tile_pool(name="psd0", space="PSUM", bufs=1) as psd0, \
             tc.tile_pool(name="psd1a", space="PSUM", bufs=1) as psd1a:
            for it in range(2):
                dots[(0, it)] = psd0.tile([128, N], f32, name=f"d0_{it}")
                dots[(1, it)] = psd1a.tile([128, N], f32, name=f"d1_{it}")

            # ---- Phase A: q/k proj -> f16 staging -> pair-packed splits,
            # dots h0+h1 it0/1 fused two pairs behind ----
            with tc.tile_pool(name="psqk", space="PSUM", bufs=1) as psqk:
                n_r = R if 1 in phases else 0
                pend_split = None
                for r in range(n_r):
                    p, par = r >> 1, r & 1
                    if 2 in phases and par == 0 and p >= 2:
                        dots_h0(p - 2, (0, 1))
                    if 2 in phases and par == 1 and p >= 4:
                        dots_h1(p - 4, (0, 1))
                    if r == 0:
                        x_sb, ri = xa0, 0
                    elif r <= 3:
                        x_sb, ri = xa1, r - 1
                    else:
                        rb, ri = divmod(r + 4, RB)
                        if ri == 0:
                            x_sb = xpool.tile([128, RB, 2, N], f16, tag="x",
                                              name=f"x1_{rb}")
                            x_dma(nc.sync, x_sb, rb * RB - 4, RB)
                    q_ps = psqk.tile([128, N], f32, tag="q")
                    k_ps = psqk.tile([128, N], f32, tag="k")
                    for cc in range(2):
                        nc.tensor.matmul(q_ps, lhsT=wq_sb[:, cc, :],
                                         rhs=x_sb[:, ri, cc, :],
                                         start=(cc == 0), stop=(cc == 1))
                    for cc in range(2):
                        nc.tensor.matmul(k_ps, lhsT=wk_sb[:, cc, :],
                                         rhs=x_sb[:, ri, cc, :],
                                         start=(cc == 0), stop=(cc == 1))
                    # stage PSUM->f16 once (q_st<-ACT, k_st<-DVE), then
                    # repack with cheap f16 copies (DVE@4x / Pool).  Splits
                    # are deferred one iteration so next r's k_st sits at
                    # DVE's queue head when k_ps stops (no WAR stall).
                    q_st = stg.tile([128, N], f16, tag="qst", name=f"qst_{r}")
                    k_st = stg.tile([128, N], f16, tag="kst", name=f"kst_{r}")
                    nc.scalar.copy(q_st, q_ps)
                    nc.vector.tensor_copy(k_st, k_ps)

                    def splits(p, par, q_st, k_st):
                        ps = slice(par * 64, (par + 1) * 64)
                        if p % 2 == 0:
                            nc.gpsimd.tensor_copy(q20[0][ps, p // 2, :], q_st[0:64, :])
                        else:
                            nc.vector.tensor_copy(q20[1][ps, p // 2, :], q_st[0:64, :])
                        nc.vector.tensor_copy(q21[p % 4][ps, p // 4, :], q_st[64:128, :])
                        nc.vector.tensor_copy(k20[p % 2][ps, p // 2, :], k_st[0:64, :])
                        nc.gpsimd.tensor_copy(k21[p % 4][ps, p // 4, :], k_st[64:128, :])

                    if pend_split is not None:
                        splits(*pend_split)
                    pend_split = (p, par, q_st, k_st)
                if pend_split is not None:
                    splits(*pend_split)
                if 2 in phases and n_r:
                    for p in (NPAIR - 2, NPAIR - 1):
                        dots_h0(p, (0, 1))
                    for p in range(NPAIR - 4, NPAIR):
                        dots_h1(p, (0, 1))

            # prefetch phase-C x blocks while phase B runs (the sync queue
            # is otherwise blocked behind softmax-gated transposes)
            x_tiles = {}
            if 3 in phases:
                for rb in range(NPREF):
                    x_tiles[rb] = xpool.tile([128, RB, 2, N], f16, tag="x",
                                             name=f"x3_{rb}")
                    nc.sync.dma_start(out=x_tiles[rb], in_=xT_blk[rb])

            # ---- Phase B1: dots h0 it2/3; it0/1 softmaxes run under ----
            with tc.tile_pool(name="psB1", space="PSUM", bufs=1) as psB1:
                for it in (2, 3):
                    dots[(0, it)] = psB1.tile([128, N], f32, name=f"d0_{it}")
                if 2 in phases:
                    for it in range(2):  # ready since end of phase A
                        transpose_attn(softmax(dots[(0, it)], 0, it), 0, it)
                        transpose_attn(softmax(dots[(1, it)], 1, it), 1, it)
                    for p in range(NPAIR):
                        dots_h0(p, (2, 3))
                    for it in (2, 3):
                        transpose_attn(softmax(dots[(0, it)], 0, it), 0, it)

            # ---- Phase B2: dots h1 it2/3; B1 softmaxes run under ----
            with tc.tile_pool(name="psB2", space="PSUM", bufs=1) as psB2:
                for it in (2, 3):
                    dots[(1, it)] = psB2.tile([128, N], f32, name=f"d1b_{it}")
                if 2 in phases:
                    for p in range(NPAIR):
                        dots_h1(p, (2, 3))
                    for it in (2, 3):
                        transpose_attn(softmax(dots[(1, it)], 1, it), 1, it)

        # -------- Phase C: v proj (2 ahead), out2+yin, y (SW pipeline) -------
        with tc.tile_pool(name="psc", space="PSUM", bufs=1) as psc, \
             tc.tile_pool(name="v2pool", bufs=3) as v2pool, \
             tc.tile_pool(name="ostpool", bufs=2) as ostpool, \
             tc.tile_pool(name="yinpool", bufs=6) as yinpool, \
             tc.tile_pool(name="ysbpool", bufs=2) as ysbpool:
            n_p = NPAIR if 3 in phases else 0
            v2s, yins = {}, {}

            # single-writer-per-tile engine map: vst2<-ACT, ost0<-ACT,
            # ost1<-DVE, yin0<-Pool, yin1<-DVE, ysb0<-ACT, ysb1<-DVE.
            # out2's lhsT reads vst2 directly with a strided AP (par is a
            # free-dim there), so no pair-repack copies are needed at all.
            def stage_a(p):
                # free layout (jc, h, par, d): the out2 lhsT slice
                # [:, jc, h] merges to one contiguous 128-wide free dim
                # (walrus rejects multi-free-dim matmul operands)
                vst2 = v2pool.tile([128, 4, 2, 2, 64], f16, tag="vst2",
                                   name=f"vst2_{p}")
                v2s[p] = vst2
                for par in range(2):
                    r = 2 * p + par
                    rb, ri = divmod(r, RB)
                    if ri == 0 and rb >= NPREF:
                        x_tiles[rb] = xpool.tile([128, RB, 2, N], f16, tag="x",
                                                 name=f"x3_{rb}")
                        nc.sync.dma_start(out=x_tiles[rb], in_=xT_blk[rb])
                    x_sb = x_tiles[rb]
                    v_ps = psv.tile([128, 4, HP], f32, tag="v",
                                    name=f"v_ps_{r}")
                    for jt in range(4):
                        for cc in range(2):
                            nc.tensor.matmul(
                                v_ps[:, jt, :],
                                lhsT=x_sb[:, ri, cc, jt * 128:(jt + 1) * 128],
                                rhs=wv_sb[:, cc, :],
                                start=(cc == 0), stop=(cc == 1))
                    dst = vst2[:, :, :, par, :]
                    vsrc = v_ps.rearrange("p a (h d) -> p a h d", h=2)
                    if par == 0:
                        nc.scalar.copy(dst, vsrc)
                    else:
                        # DVE queue-head: vst2 completes ~0.7us sooner; the
                        # WAW behind ACT's par-0 copy is naturally satisfied
                        nc.vector.tensor_copy(dst, vsrc)

            def stage_b(p):
                vst2 = v2s.pop(p)
                o = [psc.tile([128, N], f32, tag=f"o{h}", bufs=1 + h,
                              name=f"o{h}_{p}") for h in range(2)]
                for h in range(2):
                    for jc in range(4):
                        nc.tensor.matmul(
                            o[h],
                            lhsT=vst2[:, jc, h],
                            rhs=attnT[h][:, jc, :],
                            start=(jc == 0), stop=(jc == 3))
                ost = [ostpool.tile([128, N], f16, tag=f"ost{h}",
                                    name=f"ost{h}_{p}") for h in range(2)]
                nc.scalar.copy(ost[0], o[0])
                nc.vector.tensor_copy(ost[1], o[1])
                yin = [yinpool.tile([128, N], f16, tag="yin",
                                    name=f"yin_{p}_{par}") for par in range(2)]
                nc.gpsimd.tensor_copy(yin[0][0:64, :], ost[0][0:64, :])
                nc.vector.tensor_copy(yin[1][0:64, :], ost[0][64:128, :])
                nc.gpsimd.tensor_copy(yin[0][64:128, :], ost[1][0:64, :])
                nc.vector.tensor_copy(yin[1][64:128, :], ost[1][64:128, :])
                yins[p] = yin

            def stage_c(p, par):
                yin = yins[p]
                if True:
                    ysb = ysbpool.tile([128, 4, E], f16, tag=f"ysb{par}",
                                       name=f"ysb{par}_{p}")
                    # y in half-tiles: 4x 1-bank rotation (bufs=3) frees a
                    # PSUM bank for o1's double-buffer
                    for hf in range(2):
                        y_ps = psc.tile([128, 2, E], f32, tag="y", bufs=3,
                                        name=f"y_ps_{p}_{par}_{hf}")
                        for it in range(2):
                            i4 = hf * 2 + it
                            nc.tensor.matmul(
                                y_ps[:, it, :],
                                lhsT=yin[par][:, i4 * 128:(i4 + 1) * 128],
                                rhs=wo_sb, start=True, stop=True)
                        hs = slice(hf * 2, (hf + 1) * 2)
                        if par == 0:
                            nc.scalar.copy(ysb[:, hs, :], y_ps)
                        else:
                            nc.vector.tensor_copy(ysb[:, hs, :], y_ps)
                    if par == 1 and p == NPAIR - 1:
                        nc.scalar.dma_start(out=y_blk[p][:, par, 0:2],
                                            in_=ysb[:, 0:2])
                        nc.sync.dma_start(out=y_blk[p][:, par, 2:4],
                                          in_=ysb[:, 2:4])
                    else:
                        nc.scalar.dma_start(out=y_blk[p][:, par], in_=ysb)
                if par == 1:
                    yins.pop(p)

            for s in range(n_p + 5):
                if s < n_p:
                    stage_a(s)
                if 0 <= s - 2 < n_p:
                    stage_b(s - 2)
                if 0 <= s - 3 < n_p:
                    stage_c(s - 3, 0)
                if 0 <= s - 4 < n_p:
                    stage_c(s - 4, 1)

    return nc


def _get_program():
    if "nc" not in _CACHE:
        nc = build_program()
        nc.finalize()
        _CACHE["nc"] = nc
    return _CACHE["nc"]


def make_in_maps(x, Wq, Wkv, Wo):
    """Host-side sharding: core = bi*4 + hpi."""
    scale = (64.0 ** -0.5) * (64.0 ** -0.5)
    x = np.asarray(x, np.float32)
    Wq = np.asarray(Wq, np.float32) * np.float32(scale)
    Wkv = np.asarray(Wkv, np.float32)
    Wo = np.asarray(Wo, np.float32)
    b = x.shape[0] // R
    xT = np.ascontiguousarray(
        x.reshape(b, R, N, C).transpose(0, 1, 3, 2)).astype(np.float16)
    in_maps = []
    for core in range(NCORES):
        bi, hpi = divmod(core, 4)
        cols = slice(hpi * HP, (hpi + 1) * HP)
        in_maps.append({
            "xT": xT[bi],
            "wq": np.ascontiguousarray(Wq[:, cols]).astype(np.float16),
            "wk": np.ascontiguousarray(Wkv[:, cols]).astype(np.float16),
            "wv": np.ascontiguousarray(
                Wkv[:, 512 + hpi * HP: 512 + (hpi + 1) * HP]).astype(np.float16),
            "wo": np.ascontiguousarray(Wo[cols, :]).astype(np.float16),
        })
    return in_maps


def combine_outputs(ys, bo):
    """ys: list of 8 [R, N, E] f16 partials in core order; -> [B, n, dim]."""
    y0 = ys[0].astype(np.float32) + ys[1] + ys[2] + ys[3]
    y1 = ys[4].astype(np.float32) + ys[5] + ys[6] + ys[7]
    y = np.concatenate([y0, y1], axis=0).reshape(2 * R, N, E)
    return (y + np.asarray(bo, np.float32)).astype(np.float32)


def kernel(x, Wq, Wkv, Wo, bo, tie_attn_dim):
    assert int(tie_attn_dim) == R, f"hardcoded for tie_attn_dim={R}"
    from concourse.bass_utils import run_bass_kernel_spmd

    nc = _get_program()
    in_maps = make_in_maps(x, Wq, Wkv, Wo)
    res = run_bass_kernel_spmd(nc, in_maps, list(range(NCORES)))
    ys = [np.asarray(res.results[c]["y"]) for c in range(NCORES)]
    return combine_outputs(ys, bo)


# revision 70
# speedup vs baseline: 1.0026x; 1.0026x over previous
"""Tied-row (MSA) attention on 8 Trainium2 NeuronCores.

Reference computation (B=128, n=512, dim=256, h=8, dh=64,
r=tie_attn_dim=64, b=B//r=2):
    q = x @ Wq ; k,v = split(x @ Wkv)
    dots[b,h,i,j] = sum_{r,d} q[b,r,h,i,d] k[b,r,h,j,d] * scale
    attn = softmax_j(dots)
    out[b,r,h,i,d] = sum_j attn[b,h,i,j] v[b,r,h,j,d]
    y = out @ Wo + bo
Sharding: 8 cores = b(2) x head-pairs(4).  Each core owns one batch
element and 2 of the 8 heads and produces the f16 partial
    y_part = out[:, :, own 2 heads, :] @ Wo[own 128 rows, :]
The host sums the 4 partials per b and adds bo.

Design notes (driven by the TimelineSim cost model):
- A matmul is charged free_size(out) cycles regardless of contraction
  depth K or output-partition count P, so every matmul is arranged as
  K=128 x P=128: total PE time is the MAC floor, 524288 rows ~= 219 us
  (the naive K=64/P=64 layout costs 786432).
- The K=128 contractions need pair-packed layouts [(par,d), ...] where
  par = r&1: built by repacking copies.  GPSIMD cannot touch PSUM, so
  each PSUM result is staged once to f16 SBUF (ACT/DVE), then repacked
  with f16 SBUF->SBUF copies (DVE runs those at 4x; Pool is legal too).
- Per-tile single-writer-engine discipline everywhere: a second writer
  on a different engine serializes behind the first (cross-engine WAW
  in Tile's tracker), which otherwise poisons the WAR chains.

Layouts (pair = r>>1):
  q20/k20[m]  [128=(par,d), 16, 512] f16, m = pair%2 (parity-split so
              the fused dots' tile-granular RAW dep is 2 pairs stale)
  q21/k21[m]  [128=(par,d), 8, 512] f16, m = pair%4 (4-way: the h1 dots
              run 4 pairs behind; their k21 supply chain is DVE->Pool)
  dots[h,it]  [128 i, 512 j] PSUM, accumulated over all 32 pairs
  attnT[h]    [128 j, 4 jc, 512 i] f16 via xbar DMA transposes
  vst2        [128 j, 4 jc, 2 h, 2 par, 64 d] f16 per pair (out2's lhsT
              slices [:, jc, h] read it directly; par is a free dim there
              so no partition repack is needed, unlike q/k)
  o[h]        [128=(par,d), 512 i] PSUM = sum_jc vst2^T @ attnT (K=128)
  yin[par]    [128=(h,d), 512 i] f16 (f16 splits regroup (par,d)->(h,d))
  y_ps        [128 i, 4 it, 256 e] PSUM = yin^T @ wo (K=128)

Phase-A engine map (per r, 1278ns PE budget): ACT: q_st stage + q20
split on even pairs; DVE: k_st stage + q21/k20 splits (+q20 odd pairs);
Pool: k21 split.  Phase-C map (per pair, 3413ns): ACT: vst2 par0 +
ost0 + ysb0; DVE: vst2 par1 + ost1 + yin1 + ysb1; Pool: yin0.  The y
matmuls run in 1-bank half-tiles (bufs=3) so o1 can double-buffer, and
the par-1 y stage lags one slot behind par-0.

Phases: A) r-loop: q/k proj, stage, split; dots h0 it0/1 fused two
pairs behind and h1 it0/1 four pairs behind (PSUM: psv 2 + d0 2 + d1a 2
+ qk 2).  B1) dots h0 it2/3 + early softmaxes; B2) dots h1 it2/3 + the
rest; phase-C x blocks prefetch during B; psv keeps v_ps banks free of
any softmax dependency.  C) pair-loop SW pipeline a=v-proj (2 ahead),
b=out2+yin, c=y+DMA (last-pair drain split across engines/queues).

Built with bacc.Bacc(): its compile() pass legalizes Tile's sync for
walrus; callers must finalize() before running (see _get_program).
"""

import os
import sys

for _p in ("/opt/trn_rl_repo", "/root/.axon_site/_ro/trn_rl_repo"):
    if os.path.isdir(_p) and _p not in sys.path:
        sys.path.insert(0, _p)

import numpy as np

R = 64          # tie dim (MSA rows per batch element)
NPAIR = 32      # r-pairs
RB = 4          # rows per x DMA block
NPREF = 3       # phase-C x blocks prefetched before phase B
N = 512         # sequence length
C = 256         # model dim
HP = 128        # head-pair width: 2 heads x 64
E = 256         # output dim
NCORES = 8

_CACHE = {}


def build_program(phases=(1, 2, 3)):
    import concourse.bacc as bacc
    from concourse import mybir
    from concourse.tile import TileContext
    from contextlib import ExitStack

    f32 = mybir.dt.float32
    f16 = mybir.dt.float16

    nc = bacc.Bacc()
    xT = nc.declare_dram_parameter("xT", [R, C, N], f16, isOutput=False)
    wq = nc.declare_dram_parameter("wq", [C, HP], f16, isOutput=False)
    wk = nc.declare_dram_parameter("wk", [C, HP], f16, isOutput=False)
    wv = nc.declare_dram_parameter("wv", [C, HP], f16, isOutput=False)
    wo = nc.declare_dram_parameter("wo", [HP, E], f16, isOutput=False)
    y = nc.declare_dram_parameter("y", [R, N, E], f16, isOutput=True)

    # x block rb viewed as [p, r_in_block, c_chunk, n]
    xT_blk = xT.rearrange("(rb r) (cc p) n -> rb p r cc n", r=RB, p=128)
    # y block per pair: [p=i, par, i_tile, e]
    y_blk = y.rearrange("(pair par) (t p) e -> pair p par t e", par=2, p=128)

    with TileContext(nc) as tc, ExitStack() as ctx:
        singles = ctx.enter_context(tc.tile_pool(name="singles", bufs=1))
        sm = ctx.enter_context(tc.tile_pool(name="sm", bufs=4))
        attnp = ctx.enter_context(tc.tile_pool(name="attnp", bufs=4))
        attntp = ctx.enter_context(tc.tile_pool(name="attntp", bufs=2))
        xpool = ctx.enter_context(tc.tile_pool(name="xpool", bufs=NPREF))
        stg = ctx.enter_context(tc.tile_pool(name="stg", bufs=2))
        resid = ctx.enter_context(tc.tile_pool(name="resid", bufs=1))

        # first x rows go out before anything else so phase A starts ASAP;
        # row blocks 1|3|4|4|... so the first q matmul waits on a 1-row DMA
        def x_dma(engine, tile, r0, nr):
            engine.dma_start(
                out=tile,
                in_=xT[r0:r0 + nr].rearrange("r (cc p) n -> p r cc n", p=128))

        xa0 = xpool.tile([128, 1, 2, N], f16, tag="x", name="x1_h0")
        x_dma(nc.sync, xa0, 0, 1)

        # weights: [256, X] -> sbuf [128, 2, X]; one DMA each on the HWDGE
        # queues, emitted after the first x rows so they queue behind them
        # on DMA_ENGINES
        wq_sb = singles.tile([128, 2, HP], f16)
        wk_sb = singles.tile([128, 2, HP], f16)
        wv_sb = singles.tile([128, 2, HP], f16)
        wo_sb = singles.tile([128, E], f16)
        nc.scalar.dma_start(out=wq_sb, in_=wq.rearrange("(cc p) h -> p cc h", p=128))
        xa1 = xpool.tile([128, 3, 2, N], f16, tag="x", name="x1_h1")
        x_dma(nc.scalar, xa1, 1, 3)
        nc.sync.dma_start(out=wk_sb, in_=wk.rearrange("(cc p) h -> p cc h", p=128))
        nc.scalar.dma_start(out=wv_sb, in_=wv.rearrange("(cc p) h -> p cc h", p=128))
        nc.sync.dma_start(out=wo_sb, in_=wo[:, :])

        # pair-packed projections, resident through phases A+B
        q20 = [resid.tile([128, NPAIR // 2, N], f16, name=f"q20_{m}")
               for m in range(2)]
        k20 = [resid.tile([128, NPAIR // 2, N], f16, name=f"k20_{m}")
               for m in range(2)]
        q21 = [resid.tile([128, NPAIR // 4, N], f16, name=f"q21_{m}")
               for m in range(4)]
        k21 = [resid.tile([128, NPAIR // 4, N], f16, name=f"k21_{m}")
               for m in range(4)]
        # attnT survives into phase C
        attnT = [attntp.tile([128, 4, N], f16, tag="attnT", name=f"attnT_{h}")
                 for h in range(2)]

        def softmax(dots_hit, h, it):
            """dots PSUM tile -> normalized f16 attn SBUF tile.

            No max-subtraction: dots = q k^T with the 1/(sqrt(dh) sqrt(r))
            scale folded into Wq, so entries are ~N(0,1) and exp cannot
            overflow fp32/fp16."""
            ssum = sm.tile([128, 1], f32, tag="ssum", bufs=8)
            rinv = sm.tile([128, 1], f32, tag="rinv", bufs=8)
            attn = attnp.tile([128, N], f16, tag="attn", bufs=4,
                              name=f"attn_{h}_{it}")
            nc.scalar.activation(
                out=attn, in_=dots_hit,
                func=mybir.ActivationFunctionType.Exp,
                accum_out=ssum)
            nc.vector.reciprocal(rinv, ssum)
            nc.vector.tensor_scalar_mul(attn, attn, rinv)
            return attn

        def transpose_attn(attn, h, it):
            # one f16 xbar DMA transpose, SBUF -> SBUF: out[j, jc, i] =
            # attn[i, jc*128 + j]
            nc.sync.dma_start_transpose(
                out=attnT[h][:, :, it * 128:(it + 1) * 128], in_=attn)

        dots = {}

        def dots_h0(p, its):
            qq, kk, j = q20[p % 2], k20[p % 2], p // 2
            for it in its:
                nc.tensor.matmul(
                    dots[(0, it)],
                    lhsT=qq[:, j, it * 128:(it + 1) * 128], rhs=kk[:, j, :],
                    start=(p == 0), stop=(p == NPAIR - 1))

        def dots_h1(p, its):
            qq, kk, j = q21[p % 4], k21[p % 4], p // 4
            for it in its:
                nc.tensor.matmul(
                    dots[(1, it)],
                    lhsT=qq[:, j, it * 128:(it + 1) * 128], rhs=kk[:, j, :],
                    start=(p == 0), stop=(p == NPAIR - 1))

        # PSUM plan (8 banks, LIFO): psv(2, v_ps — reserved from t=0 so
        # phase-C v-projs never wait on a softmax) > psd0(2, dots h0 it0/1)
        # > psd1a(2, dots h1 it0/1) > [psqk(2) | psB1(2, h0 it2/3) |
        # psB2(2, h1 it2/3)]; after popping psB2/psd1a/psd0: psc(6).
        psv = ctx.enter_context(tc.tile_pool(name="psv", space="PSUM", bufs=2))
        with tc.tile_pool(name="psd0", space="PSUM", bufs=1) as psd0, \
             tc.tile_pool(name="psd1a", space="PSUM", bufs=1) as psd1a:
            for it in range(2):
                dots[(0, it)] = psd0.tile([128, N], f32, name=f"d0_{it}")
                dots[(1, it)] = psd1a.tile([128, N], f32, name=f"d1_{it}")

            # ---- Phase A: q/k proj -> f16 staging -> pair-packed splits,
            # dots h0+h1 it0/1 fused two pairs behind ----
            with tc.tile_pool(name="psqk", space="PSUM", bufs=1) as psqk:
                n_r = R if 1 in phases else 0
                pend_split = None
                for r in range(n_r):
                    p, par = r >> 1, r & 1
                    if 2 in phases and par == 0 and p >= 2:
                        dots_h0(p - 2, (0, 1))
                    if 2 in phases and par == 1 and p >= 4:
                        dots_h1(p - 4, (0, 1))
                    if r == 0:
                        x_sb, ri = xa0, 0
                    elif r <= 3:
                        x_sb, ri = xa1, r - 1
                    else:
                        rb, ri = divmod(r + 4, RB)
                        if ri == 0:
                            x_sb = xpool.tile([128, RB, 2, N], f16, tag="x",
                                              name=f"x1_{rb}")
                            x_dma(nc.sync, x_sb, rb * RB - 4, RB)
                    q_ps = psqk.tile([128, N], f32, tag="q")
                    k_ps = psqk.tile([128, N], f32, tag="k")
                    for cc in range(2):
                        nc.tensor.matmul(q_ps, lhsT=wq_sb[:, cc, :],
                                         rhs=x_sb[:, ri, cc, :],
                                         start=(cc == 0), stop=(cc == 1))
                    for cc in range(2):
                        nc.tensor.matmul(k_ps, lhsT=wk_sb[:, cc, :],
                                         rhs=x_sb[:, ri, cc, :],
                                         start=(cc == 0), stop=(cc == 1))
                    # stage PSUM->f16 once (q_st<-ACT, k_st<-DVE), then
                    # repack with cheap f16 copies (DVE@4x / Pool).  Splits
                    # are deferred one iteration so next r's k_st sits at
                    # DVE's queue head when k_ps stops (no WAR stall).
                    q_st = stg.tile([128, N], f16, tag="qst", name=f"qst_{r}")
                    k_st = stg.tile([128, N], f16, tag="kst", name=f"kst_{r}")
                    nc.scalar.copy(q_st, q_ps)
                    nc.vector.tensor_copy(k_st, k_ps)

                    def splits(p, par, q_st, k_st):
                        ps = slice(par * 64, (par + 1) * 64)
                        nc.vector.tensor_copy(q20[p % 2][ps, p // 2, :], q_st[0:64, :])
                        if p % 2 == 0:
                            # Pool only feeds head-1 tiles (4-pair slack
                            # tolerates its drift); h0 stays on DVE
                            nc.gpsimd.tensor_copy(q21[p % 4][ps, p // 4, :], q_st[64:128, :])
                        else:
                            nc.vector.tensor_copy(q21[p % 4][ps, p // 4, :], q_st[64:128, :])
                        nc.vector.tensor_copy(k20[p % 2][ps, p // 2, :], k_st[0:64, :])
                        nc.gpsimd.tensor_copy(k21[p % 4][ps, p // 4, :], k_st[64:128, :])

                    if pend_split is not None:
                        splits(*pend_split)
                    pend_split = (p, par, q_st, k_st)
                if pend_split is not None:
                    splits(*pend_split)
                if 2 in phases and n_r:
                    for p in (NPAIR - 2, NPAIR - 1):
                        dots_h0(p, (0, 1))
                    for p in range(NPAIR - 4, NPAIR):
                        dots_h1(p, (0, 1))

            # prefetch phase-C x blocks while phase B runs (the sync queue
            # is otherwise blocked behind softmax-gated transposes)
            x_tiles = {}
            if 3 in phases:
                for rb in range(NPREF):
                    x_tiles[rb] = xpool.tile([128, RB, 2, N], f16, tag="x",
                                             name=f"x3_{rb}")
                    nc.sync.dma_start(out=x_tiles[rb], in_=xT_blk[rb])

            # ---- Phase B1: dots h0 it2/3; it0/1 softmaxes run under ----
            with tc.tile_pool(name="psB1", space="PSUM", bufs=1) as psB1:
                for it in (2, 3):
                    dots[(0, it)] = psB1.tile([128, N], f32, name=f"d0_{it}")
                if 2 in phases:
                    for it in range(2):  # ready since end of phase A
                        transpose_attn(softmax(dots[(0, it)], 0, it), 0, it)
                        transpose_attn(softmax(dots[(1, it)], 1, it), 1, it)
                    for p in range(NPAIR):
                        dots_h0(p, (2, 3))
                    for it in (2, 3):
                        transpose_attn(softmax(dots[(0, it)], 0, it), 0, it)

            # ---- Phase B2: dots h1 it2/3; B1 softmaxes run under ----
            with tc.tile_pool(name="psB2", space="PSUM", bufs=1) as psB2:
                for it in (2, 3):
                    dots[(1, it)] = psB2.tile([128, N], f32, name=f"d1b_{it}")
                if 2 in phases:
                    for p in range(NPAIR):
                        dots_h1(p, (2, 3))
                    for it in (2, 3):
                        transpose_attn(softmax(dots[(1, it)], 1, it), 1, it)

        # -------- Phase C: v proj (2 ahead), out2+yin, y (SW pipeline) -------
        with tc.tile_pool(name="psc", space="PSUM", bufs=1) as psc, \
             tc.tile_pool(name="v2pool", bufs=3) as v2pool, \
             tc.tile_pool(name="ostpool", bufs=2) as ostpool, \
             tc.tile_pool(name="yinpool", bufs=6) as yinpool, \
             tc.tile_pool(name="ysbpool", bufs=2) as ysbpool:
            n_p = NPAIR if 3 in phases else 0
            v2s, yins = {}, {}

            # single-writer-per-tile engine map: vst2<-ACT, ost0<-ACT,
            # ost1<-DVE, yin0<-Pool, yin1<-DVE, ysb0<-ACT, ysb1<-DVE.
            # out2's lhsT reads vst2 directly with a strided AP (par is a
            # free-dim there), so no pair-repack copies are needed at all.
            def stage_a(p):
                # free layout (jc, h, par, d): the out2 lhsT slice
                # [:, jc, h] merges to one contiguous 128-wide free dim
                # (walrus rejects multi-free-dim matmul operands)
                vst2 = v2pool.tile([128, 4, 2, 2, 64], f16, tag="vst2",
                                   name=f"vst2_{p}")
                v2s[p] = vst2
                for par in range(2):
                    r = 2 * p + par
                    rb, ri = divmod(r, RB)
                    if ri == 0 and rb >= NPREF:
                        x_tiles[rb] = xpool.tile([128, RB, 2, N], f16, tag="x",
                                                 name=f"x3_{rb}")
                        nc.sync.dma_start(out=x_tiles[rb], in_=xT_blk[rb])
                    x_sb = x_tiles[rb]
                    v_ps = psv.tile([128, 4, HP], f32, tag="v",
                                    name=f"v_ps_{r}")
                    for jt in range(4):
                        for cc in range(2):
                            nc.tensor.matmul(
                                v_ps[:, jt, :],
                                lhsT=x_sb[:, ri, cc, jt * 128:(jt + 1) * 128],
                                rhs=wv_sb[:, cc, :],
                                start=(cc == 0), stop=(cc == 1))
                    dst = vst2[:, :, :, par, :]
                    vsrc = v_ps.rearrange("p a (h d) -> p a h d", h=2)
                    if par == 0:
                        nc.scalar.copy(dst, vsrc)
                    else:
                        # DVE queue-head: vst2 completes ~0.7us sooner; the
                        # WAW behind ACT's par-0 copy is naturally satisfied
                        nc.vector.tensor_copy(dst, vsrc)

            def stage_b(p):
                vst2 = v2s.pop(p)
                o = [psc.tile([128, N], f32, tag=f"o{h}", bufs=1 + h,
                              name=f"o{h}_{p}") for h in range(2)]
                for h in range(2):
                    for jc in range(4):
                        nc.tensor.matmul(
                            o[h],
                            lhsT=vst2[:, jc, h],
                            rhs=attnT[h][:, jc, :],
                            start=(jc == 0), stop=(jc == 3))
                ost = [ostpool.tile([128, N], f16, tag=f"ost{h}",
                                    name=f"ost{h}_{p}") for h in range(2)]
                nc.scalar.copy(ost[0], o[0])
                nc.vector.tensor_copy(ost[1], o[1])
                yin = [yinpool.tile([128, N], f16, tag="yin",
                                    name=f"yin_{p}_{par}") for par in range(2)]
                nc.gpsimd.tensor_copy(yin[0][0:64, :], ost[0][0:64, :])
                nc.vector.tensor_copy(yin[1][0:64, :], ost[0][64:128, :])
                nc.gpsimd.tensor_copy(yin[0][64:128, :], ost[1][0:64, :])
                nc.vector.tensor_copy(yin[1][64:128, :], ost[1][64:128, :])
                yins[p] = yin

            def stage_c(p, par):
                yin = yins[p]
                if True:
                    ysb = ysbpool.tile([128, 4, E], f16, tag=f"ysb{par}",
                                       name=f"ysb{par}_{p}")
                    # y in half-tiles: 4x 1-bank rotation (bufs=3) frees a
                    # PSUM bank for o1's double-buffer
                    for hf in range(2):
                        y_ps = psc.tile([128, 2, E], f32, tag="y", bufs=3,
                                        name=f"y_ps_{p}_{par}_{hf}")
                        for it in range(2):
                            i4 = hf * 2 + it
                            nc.tensor.matmul(
                                y_ps[:, it, :],
                                lhsT=yin[par][:, i4 * 128:(i4 + 1) * 128],
                                rhs=wo_sb, start=True, stop=True)
                        hs = slice(hf * 2, (hf + 1) * 2)
                        if par == 0:
                            nc.scalar.copy(ysb[:, hs, :], y_ps)
                        else:
                            nc.vector.tensor_copy(ysb[:, hs, :], y_ps)
                    if par == 1 and p == NPAIR - 1:
                        nc.scalar.dma_start(out=y_blk[p][:, par, 0:2],
                                            in_=ysb[:, 0:2])
                        nc.sync.dma_start(out=y_blk[p][:, par, 2:4],
                                          in_=ysb[:, 2:4])
                    else:
                        nc.scalar.dma_start(out=y_blk[p][:, par], in_=ysb)
                if par == 1:
                    yins.pop(p)

            for s in range(n_p + 5):
                if s < n_p:
                    stage_a(s)
                if 0 <= s - 2 < n_p:
                    stage_b(s - 2)
                if 0 <= s - 3 < n_p:
                    stage_c(s - 3, 0)
                if 0 <= s - 4 < n_p:
                    stage_c(s - 4, 1)

    return nc


def _get_program():
    if "nc" not in _CACHE:
        nc = build_program()
        nc.finalize()
        _CACHE["nc"] = nc
    return _CACHE["nc"]


def make_in_maps(x, Wq, Wkv, Wo):
    """Host-side sharding: core = bi*4 + hpi."""
    scale = (64.0 ** -0.5) * (64.0 ** -0.5)
    x = np.asarray(x, np.float32)
    Wq = np.asarray(Wq, np.float32) * np.float32(scale)
    Wkv = np.asarray(Wkv, np.float32)
    Wo = np.asarray(Wo, np.float32)
    b = x.shape[0] // R
    xT = np.ascontiguousarray(
        x.reshape(b, R, N, C).transpose(0, 1, 3, 2)).astype(np.float16)
    in_maps = []
    for core in range(NCORES):
        bi, hpi = divmod(core, 4)
        cols = slice(hpi * HP, (hpi + 1) * HP)
        in_maps.append({
            "xT": xT[bi],
            "wq": np.ascontiguousarray(Wq[:, cols]).astype(np.float16),
            "wk": np.ascontiguousarray(Wkv[:, cols]).astype(np.float16),
            "wv": np.ascontiguousarray(
                Wkv[:, 512 + hpi * HP: 512 + (hpi + 1) * HP]).astype(np.float16),
            "wo": np.ascontiguousarray(Wo[cols, :]).astype(np.float16),
        })
    return in_maps


def combine_outputs(ys, bo):
    """ys: list of 8 [R, N, E] f16 partials in core order; -> [B, n, dim]."""
    y0 = ys[0].astype(np.float32) + ys[1] + ys[2] + ys[3]
    y1 = ys[4].astype(np.float32) + ys[5] + ys[6] + ys[7]
    y = np.concatenate([y0, y1], axis=0).reshape(2 * R, N, E)
    return (y + np.asarray(bo, np.float32)).astype(np.float32)


def kernel(x, Wq, Wkv, Wo, bo, tie_attn_dim):
    assert int(tie_attn_dim) == R, f"hardcoded for tie_attn_dim={R}"
    from concourse.bass_utils import run_bass_kernel_spmd

    nc = _get_program()
    in_maps = make_in_maps(x, Wq, Wkv, Wo)
    res = run_bass_kernel_spmd(nc, in_maps, list(range(NCORES)))
    ys = [np.asarray(res.results[c]["y"]) for c in range(NCORES)]
    return combine_outputs(ys, bo)
